# revision 45
# baseline (speedup 1.0000x reference)
"""Multihead attention (B=2, S=2048, D=1024, H=16) on 8 TRN2 NeuronCores.

Sharding: core c -> batch b = c//4, head-group g = c%4 (4 heads, 256 features).
Each core computes q/k/v projections for its 256 features, attention for its
4 heads, and a row-parallel partial of the output projection. Host sums the
4 partials per batch (row-parallel TP unshard) and transposes back.

Mask gather: src_mask is per key position and ~half the keys are masked
(exp underflows to exactly 0), so the host gathers only unmasked key/value
positions, padded to NT*128 (NT=9 for ~1024 survivors). Scores, exp, AV,
k-proj and v-proj all shrink by NT/16. Padding columns get a -9e9 exp bias
so they contribute exactly 0, like masked keys did.

The run is ScalarE-bound in the middle: softmax exp is 72 ACTIVATE ops of
[128,1024] at ~1.1us each (~80us floor). Everything else is scheduled to
keep that stream fed:

DMA: all inputs are host-pre-tiled so each tensor is ONE descriptor
(descriptor issue is ~0.6us each and was the old bottleneck). Three queues
(Sync / Scalar / GpSimd), each with critical phase-1 bytes first and
phase-2 bytes queued behind them on the same queue:
  sync:   wk, xk chunks (512-col groups, all k-tiles)    | xv thirds
  scalar: consts, wq, xq i-chunks 0,1 (i 0:1024)         | xq chunks 2,3
  gpsimd: vscaf, bv, wv                                  | wo
First scores need only k/q-path phase-1 (~4.3MB) -> exp starts ~13us.

Per-core pipeline (all matmuls bf16 with f32 PSUM accumulation):
  k-proj is chunk-outer/k-inner so each 512-col chunk finishes as its xk
  descriptor lands (per-chunk bias-add frees scores j-tiles 0..3 early).
  q-proj t0 half0 before h0; q t0 half1 folded into h0-half0's j-loop
  (xq chunks 2,3 arrive mid-h0); q t1 folded into h1's j-loop. v-proj
  units folded just-in-time into h0-half0 (vproj(j) right before AV(j)).
  h0 runs its two i-halves serially (half1 depends on late xq bytes);
  h1..h3 interleave halves per j as before. Attention per head, per
  j-tile: scoresT [128 j, 1024 i] in PSUM, exp(scale*x + mask_j) fused on
  ScalarE (mask = per-partition bias), then AV with va stationary:
  po += va_j^T @ expT_j. Each head's va block is [ones | zeros | v(64)]
  so po row 0 is the softmax denominator and rows 64:128 the features.
  The divide is a DVE reciprocal, a GpSimd partition-broadcast (dst must
  start at partition 0 and src must be a separate tile), and a DVE
  multiply: odd heads write ot rows 64:128 in place, even heads go
  through a bf16 staging tile + SBUF->SBUF DMA for the partition shift.

Output projection accumulates 4 do-tiles into one [128, 4096] staging
tile and ships it as ONE 1MB descriptor, alternating sync/gpsimd queues
so the 4MB output stream overlaps the remaining compute.
"""

import math

import numpy as np

B, S, D, H = 2, 2048, 1024, 16
NCORES = 8
GH = 4                  # heads per core
HD = D // H             # 64
F = GH * HD             # 256 local features
SCALE = 1.0 / math.sqrt(HD)
NEG = np.float32(-9e9)

KT = D // 128           # 8 contraction tiles (projections)
FT = F // 128           # 2 local-feature tiles
DT = D // 128           # 8 output-feature tiles
NQC = S // 512          # 4 xq i-chunks

TRACE = False           # set by test harness; requires antenv.axon_hooks wired
LAST_EXEC_NS = None
LAST_RESULTS = None

_STATE = {}


def _chunks(width):
    c, out = 0, []
    while c < width:
        out.append((c, min(c + 512, width)))
        c += 512
    return out


def _build(nt):
    import concourse.bacc as bacc
    import concourse.mybir as mybir
    from concourse.tile import TileContext

    f32 = mybir.dt.float32
    bf16 = mybir.dt.bfloat16
    Exp = mybir.ActivationFunctionType.Exp

    SK = nt * 128               # gathered key/value length
    KCH = _chunks(SK)           # k-proj column chunks
    NV3 = (nt + 2) // 3         # xv thirds

    nc = bacc.Bacc("TRN2", target_bir_lowering=False, debug=False,
                   num_devices=NCORES)

    # host-pre-tiled inputs, one DMA descriptor each
    xq_d = [nc.declare_dram_parameter(f"xq{c}", [128, KT * 512], bf16,
                                      isOutput=False) for c in range(NQC)]
    xk_d = [nc.declare_dram_parameter(f"xk{c}", [128, KT * (c1 - c0)], bf16,
                                      isOutput=False)
            for c, (c0, c1) in enumerate(KCH)]
    xv_d = nc.declare_dram_parameter("xv2", [128, nt * D], bf16,
                                     isOutput=False)
    wq_d = nc.declare_dram_parameter("wq2", [128, KT * F], bf16, isOutput=False)
    wk_d = nc.declare_dram_parameter("wk2", [128, KT * F], bf16, isOutput=False)
    wv_d = nc.declare_dram_parameter("wv2", [128, KT * F], bf16, isOutput=False)
    wo_d = nc.declare_dram_parameter("wo2", [128, FT * D], bf16, isOutput=False)
    # packed per-partition constants: [bq(FT) | bk(FT) | bo(DT) | mask(nt)]
    cs_d = nc.declare_dram_parameter("cst", [128, FT + FT + DT + nt], f32,
                                     isOutput=False)
    bv_d = nc.declare_dram_parameter("bv", [F], bf16, isOutput=False)
    # va scaffold: zeros with a ones column per head at its parity slot
    vs_d = nc.declare_dram_parameter("vscaf", [128, GH * 128], bf16,
                                     isOutput=False)
    out_d = nc.declare_dram_parameter("out2", [128, DT, S], bf16,
                                      isOutput=True)

    with TileContext(nc) as tc:
        with tc.tile_pool(name="persist", bufs=1) as pp, \
             tc.tile_pool(name="expp", bufs=10) as ep, \
             tc.tile_pool(name="ostage", bufs=2) as osp, \
             tc.tile_pool(name="divp", bufs=2) as dp:

            def ptile(shape, dtype, name):
                return pp.tile(shape, dtype, name=name, tag=name)

            # ---- persistent SBUF tensors ----
            wq_sb = ptile([128, KT * F], bf16, "wq")
            wk_sb = ptile([128, KT * F], bf16, "wk")
            wv_sb = ptile([128, KT * F], bf16, "wv")
            wo_sb = ptile([128, FT * D], bf16, "wo")
            xq_sb = ptile([128, KT * S], bf16, "xq")
            xk_sb = ptile([128, KT * SK], bf16, "xk")
            xv_sb = ptile([128, nt * D], bf16, "xv")
            cst = ptile([128, FT + FT + DT + nt], f32, "cst")
            bq_sb = [cst[:, t:t + 1] for t in range(FT)]
            bk_sb = [cst[:, FT + t:FT + t + 1] for t in range(FT)]
            bo_sb = [cst[:, 2 * FT + d:2 * FT + d + 1] for d in range(DT)]
            mk_sb = [cst[:, 2 * FT + DT + j:2 * FT + DT + j + 1]
                     for j in range(nt)]
            bv_sb = ptile([1, F], bf16, "bvrow")
            ones_sb = ptile([1, 128], bf16, "onesrow")
            zeros_sb = ptile([128, 512], bf16, "zeros")
            vscaf = ptile([128, GH * 128], bf16, "vscaf")
            qT_sb = [ptile([128, S], bf16, f"qT{t}") for t in range(FT)]
            kT_sb = [ptile([128, SK], bf16, f"kT{t}") for t in range(FT)]
            va_sb = [ptile([128, GH * 128], bf16, f"va{j}") for j in range(nt)]
            ot_sb = [ptile([128, S], bf16, f"ot{t}") for t in range(FT)]

            nc.vector.memset(zeros_sb[:], 0.0)
            nc.vector.memset(ones_sb[:], 1.0)

            def wqs(k, t):
                return wq_sb[:, k * F + t * 128:k * F + (t + 1) * 128]

            def wks(k, t):
                return wk_sb[:, k * F + t * 128:k * F + (t + 1) * 128]

            def wvs(k):
                return wv_sb[:, k * F:(k + 1) * F]

            def wos(t, do):
                return wo_sb[:, t * D + do * 128:t * D + (do + 1) * 128]

            # xq/xk SBUF layout is CHUNK-major: [chunk][k][cols] so each DMA
            # descriptor is one fully-contiguous [128, KT*w] block on both
            # the DRAM and SBUF side (descriptor issue time scales with the
            # number of contiguous runs, so fragmented patterns cost ~8us).
            def xqs(k, c, w=512):
                o = c * KT * 512
                return xq_sb[:, o + k * w:o + (k + 1) * w]

            XKO = [0]
            for (c0, c1) in KCH:
                XKO.append(XKO[-1] + KT * (c1 - c0))

            def xks(k, ci, w):
                o = XKO[ci]
                return xk_sb[:, o + k * w:o + (k + 1) * w]

            def xvs(st, k):
                return xv_sb[:, st * D + k * 128:st * D + (k + 1) * 128]

            # ---- DMA program. Per-queue sustained throughput is only
            # ~150GB/s (aggregate ~310), so the critical path (k chunk-a +
            # q i-half0, which gate the first exp units) is split in half-
            # descriptors and balanced across ALL THREE queues; later bytes
            # queue behind them. Every descriptor is a contiguous block.
            HB = KT // 2 * 512              # half of a 512-col chunk group
            # phase-1: sync gets wk + xka-hi + xq1-hi
            nc.sync.dma_start(out=wk_sb[:], in_=wk_d[:])
            nc.sync.dma_start(out=xk_sb[:, HB:2 * HB], in_=xk_d[0][:, HB:])
            nc.sync.dma_start(out=xq_sb[:, KT * 512 + HB:2 * KT * 512],
                              in_=xq_d[1][:, HB:])
            # scalar gets consts + wq + xq0 halves + xq1-lo
            nc.scalar.dma_start(out=cst[:], in_=cs_d[:])
            nc.scalar.dma_start(out=wq_sb[:], in_=wq_d[:])
            nc.scalar.dma_start(out=xq_sb[:, 0:HB], in_=xq_d[0][:, 0:HB])
            nc.scalar.dma_start(out=xq_sb[:, HB:KT * 512],
                                in_=xq_d[0][:, HB:])
            nc.scalar.dma_start(out=xq_sb[:, KT * 512:KT * 512 + HB],
                                in_=xq_d[1][:, 0:HB])
            # gpsimd gets the tiny constants + xka-lo
            nc.gpsimd.dma_start(out=vscaf[:], in_=vs_d[:])
            nc.gpsimd.dma_start(out=bv_sb[:], in_=bv_d[:].unsqueeze(0))
            nc.gpsimd.dma_start(out=xk_sb[:, 0:HB], in_=xk_d[0][:, 0:HB])

            # phase-2: v thirds interleaved with the k tail chunks on sync
            # (vproj folds consume xv just-in-time through h0-half0), q
            # i-half1 behind on scalar, v/o weights on gpsimd.
            nc.gpsimd.dma_start(out=wv_sb[:], in_=wv_d[:])
            xvs3 = []
            for c3 in range(NV3):
                s0, s1 = c3 * 3 * D, min((c3 * 3 + 3) * D, nt * D)
                xvs3.append((s0, s1))
            nc.sync.dma_start(out=xv_sb[:, xvs3[0][0]:xvs3[0][1]],
                              in_=xv_d[:, xvs3[0][0]:xvs3[0][1]])
            for c in range(1, len(KCH)):
                nc.sync.dma_start(out=xk_sb[:, XKO[c]:XKO[c + 1]],
                                  in_=xk_d[c][:])
            for (s0, s1) in xvs3[1:]:
                nc.sync.dma_start(out=xv_sb[:, s0:s1], in_=xv_d[:, s0:s1])
            for c in range(2, NQC):
                nc.scalar.dma_start(
                    out=xq_sb[:, c * KT * 512:(c + 1) * KT * 512],
                    in_=xq_d[c][:])
            nc.gpsimd.dma_start(out=wo_sb[:], in_=wo_d[:])

            # va scaffold copies on DVE (idle until the first bias-add)
            for j in range(nt):
                nc.vector.tensor_copy(va_sb[j][:], vscaf[:])

            # PSUM: tag "pssc" = 3 rotating [128,1024] slots (6 banks) shared
            # by every transient accumulator (projections, scores, out-proj)
            # — 3 slots give the scores pipeline 2-deep lookahead so the exp
            # stream never waits. Tag "po" = 1 slot (2 banks) for the AV
            # accumulator; halves run serially in every head so only one po
            # is live at a time.
            with tc.tile_pool(name="psB", bufs=3, space="PSUM") as psB:

                def ps_tile(name, tag="pssc", bufs=3):
                    return psB.tile([128, 1024], mybir.dt.float32,
                                    name=name, tag=tag, bufs=bufs)

                # Keep-warm matmuls: the PE p-state (0.65 -> 1.2 -> 2.4GHz)
                # ramps only while the engine is continuously busy and any
                # idle resets it. During the DMA-bound head the PE would sit
                # cold, then run every projection burst at 0.65-1.2GHz.
                # These standalone zero matmuls run during the waits (no
                # data deps beyond the zeros memset) purely to hold the
                # clock up; they cost idle time only.
                def emit_warms(n):
                    w = ps_tile("warm")
                    for i in range(n):
                        nc.tensor.matmul(
                            w[:, (i % 2) * 512:(i % 2) * 512 + 512],
                            lhsT=zeros_sb[:, 0:128], rhs=zeros_sb[:],
                            start=True, stop=True)

                # zero-add into the live AV accumulator: same trick for the
                # exp-bound attention units where ~0.25us/unit of idle would
                # otherwise decay the clock. Uses the stationary the scores
                # matmul just loaded, so no weight-switch bubble.
                def fill_po(h, po_t, j):
                    ht, off = h // 2, (h % 2) * HD
                    nc.tensor.matmul(
                        po_t[:, 0:512],
                        lhsT=kT_sb[ht][off:off + HD, j * 128:(j + 1) * 128],
                        rhs=zeros_sb[off:off + HD, :],
                        start=False, stop=False, skip_group_check=True)

                # k projection, one 512-col chunk: gated only by its own xk
                # descriptor(s).
                def proj_k_chunk(ci):
                    c0, c1 = KCH[ci]
                    accs = [psB.tile([128, c1 - c0], mybir.dt.float32,
                                     name="kac", tag="pssc", bufs=3)
                            for t in range(FT)]
                    for k in range(KT):
                        for t in range(FT):
                            nc.tensor.matmul(
                                accs[t][:], lhsT=wks(k, t),
                                rhs=xks(k, ci, c1 - c0),
                                start=(k == 0), stop=(k == KT - 1))
                    for t in range(FT):
                        nc.vector.tensor_scalar_add(
                            kT_sb[t][:, c0:c1], accs[t][:], bk_sb[t])

                # q projection for one (f-tile, s-half): n-outer / k-inner so
                # chunk n is gated by xq descriptor sh*2+n only.
                def proj_q_half(t, sh, warmup=False):
                    acc = ps_tile("acc")
                    s0 = sh * 1024
                    for n in range(2):
                        if warmup:
                            emit_warms(3)
                        for k in range(KT):
                            nc.tensor.matmul(
                                acc[:, n * 512:(n + 1) * 512],
                                lhsT=wqs(k, t),
                                rhs=xqs(k, sh * 2 + n),
                                start=(k == 0), stop=(k == KT - 1))
                    nc.vector.tensor_scalar_add(
                        qT_sb[t][:, s0:s0 + 1024], acc[:], bq_sb[t])

                # v projection for one seq tile (+bias via ones-row matmul).
                # va block per head: [ones col | zeros | v(64) at cols 64:128]
                # so po row 0 is the softmax denominator, rows 64:128 the
                # features (partition_broadcast only works from row 0).
                def vproj_unit(st):
                    pv = psB.tile([128, F], mybir.dt.float32,
                                  name="pv", tag="pssc", bufs=3)
                    for k in range(KT):
                        nc.tensor.matmul(
                            pv[:], lhsT=xvs(st, k),
                            rhs=wvs(k), start=(k == 0), stop=False)
                    nc.tensor.matmul(pv[:], lhsT=ones_sb[:], rhs=bv_sb[:],
                                     start=False, stop=True)
                    for h in range(GH):
                        d0 = h * 128 + HD
                        nc.vector.tensor_copy(
                            va_sb[st][:, d0:d0 + HD],
                            pv[:, h * HD:(h + 1) * HD])

                def scores_unit(h, half, j):
                    ht, off = h // 2, (h % 2) * HD
                    i0 = half * 1024
                    ps = ps_tile("pssc")
                    for n in range(2):
                        nc.tensor.matmul(
                            ps[:, n * 512:(n + 1) * 512],
                            lhsT=kT_sb[ht][off:off + HD,
                                           j * 128:(j + 1) * 128],
                            rhs=qT_sb[ht][off:off + HD,
                                          i0 + n * 512:i0 + (n + 1) * 512],
                            start=True, stop=True)
                    e = ep.tile([128, 1024], bf16, name="expT",
                                tag="expT", bufs=10)
                    nc.scalar.activation(e[:], ps[:], Exp,
                                         bias=mk_sb[j], scale=SCALE)
                    return e

                def av_unit(h, po_t, j, e):
                    for n in range(2):
                        nc.tensor.matmul(
                            po_t[:, n * 512:(n + 1) * 512],
                            lhsT=va_sb[j][:, h * 128:(h + 1) * 128],
                            rhs=e[:, n * 512:(n + 1) * 512],
                            start=(j == 0), stop=(j == nt - 1))

                # softmax divide: po row 0 is the denominator. Optionally one
                # DVE copy evacuates PSUM so the slot frees for the next
                # head; reciprocal + partition-broadcast + multiply. Odd
                # heads (ot rows 64:128) write ot in place; even heads need
                # the partition shift via a SBUF->SBUF DMA.
                def divide(h, half, po_t, use_pox):
                    ht = h // 2
                    i0 = half * 1024
                    if use_pox:
                        pox = dp.tile([128, 1024], f32, name="pox", tag="pox")
                        nc.vector.tensor_copy(pox[:], po_t[:])
                        src = pox
                    else:
                        src = po_t
                    rec = dp.tile([1, 1024], f32, name="rec", tag="rec")
                    nc.vector.reciprocal_approx_fast(out=rec[:],
                                                     in_=src[0:1, :])
                    # NB: broadcast src must be a separate tile and the dst
                    # must start at partition 0 — ucode constraints on HW.
                    recb = dp.tile([128, 1024], f32, name="recb", tag="recb")
                    nc.gpsimd.partition_broadcast(recb[:], rec[:])
                    if h % 2 == 1:
                        nc.vector.tensor_tensor(
                            out=ot_sb[ht][HD:128, i0:i0 + 1024],
                            in0=src[HD:128, :], in1=recb[HD:128, :],
                            op=mybir.AluOpType.mult)
                    else:
                        tmp = dp.tile([128, 1024], bf16, name="tmp", tag="tmp")
                        nc.vector.tensor_tensor(
                            out=tmp[HD:128, :],
                            in0=src[HD:128, :], in1=recb[HD:128, :],
                            op=mybir.AluOpType.mult)
                        nc.sync.dma_start(
                            out=ot_sb[ht][0:HD, i0:i0 + 1024],
                            in_=tmp[HD:128, :])

                # output projection unit: one do-tile of one i-half into the
                # group staging tile; ships a 1MB descriptor after each
                # 4-do group, alternating sync/gpsimd queues. use_scalar
                # picks the bias engine (ScalarE only when it is not
                # carrying the exp stream).
                ostate = {}

                def oproj_unit(ih, do, use_scalar):
                    i0 = ih * 1024
                    dg, dl = do // 4, do % 4
                    if dl == 0:
                        ostate[(ih, dg)] = osp.tile([128, 4096], bf16,
                                                    name="stg4", tag="stg")
                    stg = ostate[(ih, dg)]
                    pso = ps_tile("pso")
                    for n in range(2):
                        for t in range(FT):
                            nc.tensor.matmul(
                                pso[:, n * 512:(n + 1) * 512],
                                lhsT=wos(t, do),
                                rhs=ot_sb[t][:, i0 + n * 512:
                                             i0 + (n + 1) * 512],
                                start=(t == 0), stop=(t == FT - 1))
                    if use_scalar and do % 2 == 1:
                        nc.scalar.add(stg[:, dl * 1024:(dl + 1) * 1024],
                                      pso[:], bo_sb[do])
                    else:
                        nc.vector.tensor_scalar_add(
                            stg[:, dl * 1024:(dl + 1) * 1024],
                            pso[:], bo_sb[do])
                    if dl == 3:
                        eng = nc.sync if dg == 0 else nc.gpsimd
                        eng.dma_start(
                            out=out_d[:, dg * 4:(dg + 1) * 4, i0:i0 + 1024],
                            in_=stg[:].rearrange("p (d i) -> p d i", d=4))

                # ---------------- emission schedule ----------------
                # chunk-a of k and the q i-half0 projection gate the first
                # exp; the k tail chunks (b, c) have later-arriving data and
                # are emitted after so they can't block the first scores.
                emit_warms(12)
                proj_k_chunk(0)
                proj_q_half(0, 0, warmup=True)
                for ci in range(1, len(KCH)):
                    proj_k_chunk(ci)

                # every head runs its two i-halves serially (one exp unit
                # per j). Folds fill the exp-wait bubbles with real work:
                # h0-half0 carries the v-proj units (one per j, just-in-time
                # for its AV) and the q t0-half1 burst near the end (xq
                # chunks 2,3 arrive mid-loop); h1 carries the q t1 bursts.
                for h in range(GH):
                    for half in range(2):
                        po_t = psB.tile([128, 1024], mybir.dt.float32,
                                        name="po", tag="po", bufs=1)
                        for j in range(nt):
                            e = scores_unit(h, half, j)
                            folded = False
                            if h == 0 and half == 0:
                                vproj_unit(j)
                                folded = True
                                if j == max(nt - 3, 1):
                                    proj_q_half(0, 1)
                            if h == 1 and j == min(1, nt - 1):
                                proj_q_half(1, half)
                                folded = True
                            if not folded and j > 0:
                                fill_po(h, po_t, j)
                            av_unit(h, po_t, j, e)
                        last = (h == GH - 1 and half == 1)
                        divide(h, half, po_t, use_pox=not last)
                        if last:
                            # ot half0 completed back at divide(h3,0), so
                            # out_proj(0) streams immediately after the last
                            # AV while the half1 divide chain runs on DVE/
                            # GpSimd; out_proj(1) follows as ot half1 lands.
                            for do in range(DT):
                                oproj_unit(0, do, use_scalar=True)
                            for do in range(DT):
                                oproj_unit(1, do, use_scalar=True)

    nc.compile()
    return nc


def kernel(query, key, value, src_mask, Wq, bq, Wk, bk, Wv, bv, Wo, bo, nhead):
    global LAST_EXEC_NS, LAST_RESULTS
    import ml_dtypes
    from concourse.bass_utils import run_bass_kernel_spmd

    assert int(nhead) == H
    bf16 = ml_dtypes.bfloat16
    query = np.asarray(query, dtype=np.float32)
    key = np.asarray(key, dtype=np.float32)
    value = np.asarray(value, dtype=np.float32)
    src_mask = np.asarray(src_mask)
    Wq, bq = np.asarray(Wq, np.float32), np.asarray(bq, np.float32)
    Wk, bk = np.asarray(Wk, np.float32), np.asarray(bk, np.float32)
    Wv, bv = np.asarray(Wv, np.float32), np.asarray(bv, np.float32)
    Wo, bo = np.asarray(Wo, np.float32), np.asarray(bo, np.float32)

    # gather unmasked key/value positions (masked keys contribute exactly 0)
    idxs = [np.flatnonzero(~src_mask[b]) for b in range(B)]
    nt = max(1, (max(len(ix) for ix in idxs) + 127) // 128)
    SK = nt * 128
    KCH = _chunks(SK)

    if nt not in _STATE:
        _STATE[nt] = _build(nt)
    nc = _STATE[nt]

    def tile_p(mat2d):
        # [KT*128, W] -> [128, KT, W] (partition-major k-tiling)
        w = mat2d.shape[1]
        return np.ascontiguousarray(
            mat2d.reshape(KT, 128, w).transpose(1, 0, 2))

    xq_c, xk_c, xv2, maskf = [], [], [], []
    for b in range(B):
        qt = tile_p(query[b].T.astype(bf16))      # [128, KT, S]
        xq_c.append([np.ascontiguousarray(
            qt[:, :, c * 512:(c + 1) * 512]).reshape(128, -1)
            for c in range(NQC)])
        ix = idxs[b]
        nu = len(ix)
        kg = np.zeros((SK, D), np.float32)
        kg[:nu] = key[b][ix]
        kt = tile_p(kg.T.astype(bf16))            # [128, KT, SK]
        xk_c.append([np.ascontiguousarray(kt[:, :, c0:c1]).reshape(128, -1)
                     for (c0, c1) in KCH])
        vg = np.zeros((SK, D), np.float32)
        vg[:nu] = value[b][ix]
        # xv2[p, st*D + k*128+c] = vg.T[k*128+p, st*128+c]
        xv2.append(np.ascontiguousarray(
            vg.T.reshape(KT, 128, nt, 128).transpose(1, 2, 0, 3)
            .reshape(128, nt * D)).astype(bf16))
        mk = np.where(np.arange(SK) < nu, np.float32(0), NEG)
        maskf.append(np.ascontiguousarray(
            mk.reshape(nt, 128).T.astype(np.float32)))

    # va scaffold: ones column at the head block start (denominator row 0)
    vscaf = np.zeros((128, GH * 128), np.float32)
    for h in range(GH):
        vscaf[:, h * 128] = 1.0
    vscaf = vscaf.astype(bf16)

    wq2, wk2, wv2, wo2, cst, bvs = [], [], [], [], [], []
    for g in range(NCORES // B):
        gs, ge = g * F, (g + 1) * F
        wq2.append(np.ascontiguousarray(
            tile_p(Wq[gs:ge, :].T.astype(bf16)).reshape(128, KT * F)))
        wk2.append(np.ascontiguousarray(
            tile_p(Wk[gs:ge, :].T.astype(bf16)).reshape(128, KT * F)))
        wv2.append(np.ascontiguousarray(
            tile_p(Wv[gs:ge, :].T.astype(bf16)).reshape(128, KT * F)))
        # wo2[p, t*D+c] = Wo[:, gs:ge].T[t*128+p, c]
        woT = Wo[:, gs:ge].T.astype(bf16)          # [F, D]
        wo2.append(np.ascontiguousarray(
            woT.reshape(FT, 128, D).transpose(1, 0, 2).reshape(128, FT * D)))
        bq2 = bq[gs:ge].reshape(FT, 128).T
        bk2 = bk[gs:ge].reshape(FT, 128).T
        bvs.append(bv[gs:ge].astype(bf16))
        bo2 = bo.reshape(DT, 128).T if g == 0 else np.zeros((128, DT),
                                                            np.float32)
        cst.append((bq2, bk2, bo2))

    in_maps = []
    for c in range(NCORES):
        b, g = c // (NCORES // B), c % (NCORES // B)
        bq2, bk2, bo2 = cst[g]
        cpack = np.ascontiguousarray(np.concatenate(
            [bq2, bk2, bo2, maskf[b]], axis=1).astype(np.float32))
        m = {"xv2": xv2[b], "wq2": wq2[g], "wk2": wk2[g], "wv2": wv2[g],
             "wo2": wo2[g], "cst": cpack, "bv": bvs[g], "vscaf": vscaf}
        for ci in range(NQC):
            m[f"xq{ci}"] = xq_c[b][ci]
        for ci in range(len(KCH)):
            m[f"xk{ci}"] = xk_c[b][ci]
        in_maps.append(m)

    kwargs = {}
    if TRACE:
        kwargs = dict(trace=True)
    res = run_bass_kernel_spmd(nc, in_maps, core_ids=list(range(NCORES)),
                               **kwargs)
    LAST_EXEC_NS = res.exec_time_ns
    LAST_RESULTS = res

    out = np.empty((B, S, D), dtype=np.float32)
    for b in range(B):
        acc = res.results[b * (NCORES // B)]["out2"].astype(np.float32)
        for g in range(1, NCORES // B):
            acc = acc + res.results[b * (NCORES // B) + g]["out2"]
        # out2 [128, DT, S] -> [D, S] -> [S, D]
        out[b] = acc.transpose(1, 0, 2).reshape(D, S).T
    return out


# revision 49
# speedup vs baseline: 1.1290x; 1.1290x over previous
"""Multihead attention (B=2, S=2048, D=1024, H=16) on 8 TRN2 NeuronCores.

Sharding: core c -> batch b = c//4, head-group g = c%4 (4 heads, 256 features).
Each core computes q/k/v projections for its 256 features, attention for its
4 heads, and a row-parallel partial of the output projection. Host sums the
4 partials per batch (row-parallel TP unshard) and transposes back.

Mask gather: src_mask is per key position and ~half the keys are masked
(exp underflows to exactly 0), so the host gathers only unmasked key/value
positions, padded to NT*128 (NT=9 for ~1024 survivors). Scores, exp, AV,
k-proj and v-proj all shrink by NT/16. Padding columns get a -9e9 exp bias
so they contribute exactly 0, like masked keys did.

The run is ScalarE-bound in the middle: softmax exp is 72 ACTIVATE ops of
[128,1024] at ~1.1us each (~80us floor). Everything else is scheduled to
keep that stream fed:

DMA: all inputs are host-pre-tiled so each tensor is ONE descriptor
(descriptor issue is ~0.6us each and was the old bottleneck). Three queues
(Sync / Scalar / GpSimd), each with critical phase-1 bytes first and
phase-2 bytes queued behind them on the same queue:
  sync:   wk, xk chunks (512-col groups, all k-tiles)    | xv thirds
  scalar: consts, wq, xq i-chunks 0,1 (i 0:1024)         | xq chunks 2,3
  gpsimd: vscaf, bv, wv                                  | wo
First scores need only k/q-path phase-1 (~4.3MB) -> exp starts ~13us.

Per-core pipeline (all matmuls bf16 with f32 PSUM accumulation):
  k-proj is chunk-outer/k-inner so each 512-col chunk finishes as its xk
  descriptor lands (per-chunk bias-add frees scores j-tiles 0..3 early).
  q-proj t0 half0 before h0; q t0 half1 folded into h0-half0's j-loop
  (xq chunks 2,3 arrive mid-h0); q t1 folded into h1's j-loop. v-proj
  units folded just-in-time into h0-half0 (vproj(j) right before AV(j)).
  h0 runs its two i-halves serially (half1 depends on late xq bytes);
  h1..h3 interleave halves per j as before. Attention per head, per
  j-tile: scoresT [128 j, 1024 i] in PSUM, exp(scale*x + mask_j) fused on
  ScalarE (mask = per-partition bias), then AV with va stationary:
  po += va_j^T @ expT_j. Each head's va block is [ones | zeros | v(64)]
  so po row 0 is the softmax denominator and rows 64:128 the features.
  The divide is a DVE reciprocal, a GpSimd partition-broadcast (dst must
  start at partition 0 and src must be a separate tile), and a DVE
  multiply: odd heads write ot rows 64:128 in place, even heads go
  through a bf16 staging tile + SBUF->SBUF DMA for the partition shift.

Output projection accumulates 4 do-tiles into one [128, 4096] staging
tile and ships it as ONE 1MB descriptor, alternating sync/gpsimd queues
so the 4MB output stream overlaps the remaining compute.
"""

import math

import numpy as np

B, S, D, H = 2, 2048, 1024, 16
NCORES = 8
GH = 4                  # heads per core
HD = D // H             # 64
F = GH * HD             # 256 local features
SCALE = 1.0 / math.sqrt(HD)
NEG = np.float32(-9e9)

KT = D // 128           # 8 contraction tiles (projections)
FT = F // 128           # 2 local-feature tiles
DT = D // 128           # 8 output-feature tiles
NQC = S // 512          # 4 xq i-chunks

TRACE = False           # set by test harness; requires antenv.axon_hooks wired
LAST_EXEC_NS = None
LAST_RESULTS = None

_STATE = {}


def _chunks(width):
    c, out = 0, []
    while c < width:
        out.append((c, min(c + 512, width)))
        c += 512
    return out


def _build(nt):
    import concourse.bacc as bacc
    import concourse.mybir as mybir
    from concourse.tile import TileContext

    f32 = mybir.dt.float32
    bf16 = mybir.dt.bfloat16
    Exp = mybir.ActivationFunctionType.Exp

    SK = nt * 128               # gathered key/value length
    KCH = _chunks(SK)           # k-proj column chunks
    NV3 = (nt + 2) // 3         # xv thirds

    nc = bacc.Bacc("TRN2", target_bir_lowering=False, debug=False,
                   num_devices=NCORES)

    # host-pre-tiled inputs, one DMA descriptor each
    xq_d = [nc.declare_dram_parameter(f"xq{c}", [128, KT * 512], bf16,
                                      isOutput=False) for c in range(NQC)]
    xk_d = [nc.declare_dram_parameter(f"xk{c}", [128, KT * (c1 - c0)], bf16,
                                      isOutput=False)
            for c, (c0, c1) in enumerate(KCH)]
    xv_d = nc.declare_dram_parameter("xv2", [128, nt * D], bf16,
                                     isOutput=False)
    wq_d = nc.declare_dram_parameter("wq2", [128, KT * F], bf16, isOutput=False)
    wk_d = nc.declare_dram_parameter("wk2", [128, KT * F], bf16, isOutput=False)
    wv_d = nc.declare_dram_parameter("wv2", [128, KT * F], bf16, isOutput=False)
    wo_d = nc.declare_dram_parameter("wo2", [128, FT * D], bf16, isOutput=False)
    # packed per-partition constants: [bq(FT) | bk(FT) | bo(DT) | mask(nt)]
    cs_d = nc.declare_dram_parameter("cst", [128, FT + FT + DT + nt], f32,
                                     isOutput=False)
    bv_d = nc.declare_dram_parameter("bv", [F], bf16, isOutput=False)
    # va scaffold: zeros with a ones column per head at its parity slot
    vs_d = nc.declare_dram_parameter("vscaf", [128, GH * 128], bf16,
                                     isOutput=False)
    out_d = nc.declare_dram_parameter("out2", [128, DT, S], bf16,
                                      isOutput=True)

    with TileContext(nc) as tc:
        with tc.tile_pool(name="persist", bufs=1) as pp, \
             tc.tile_pool(name="expp", bufs=10) as ep, \
             tc.tile_pool(name="ostage", bufs=2) as osp, \
             tc.tile_pool(name="divp", bufs=2) as dp:

            def ptile(shape, dtype, name):
                return pp.tile(shape, dtype, name=name, tag=name)

            # ---- persistent SBUF tensors ----
            wq_sb = ptile([128, KT * F], bf16, "wq")
            wk_sb = ptile([128, KT * F], bf16, "wk")
            wv_sb = ptile([128, KT * F], bf16, "wv")
            wo_sb = ptile([128, FT * D], bf16, "wo")
            xq_sb = ptile([128, KT * S], bf16, "xq")
            xk_sb = ptile([128, KT * SK], bf16, "xk")
            xv_sb = ptile([128, nt * D], bf16, "xv")
            cst = ptile([128, FT + FT + DT + nt], f32, "cst")
            bq_sb = [cst[:, t:t + 1] for t in range(FT)]
            bk_sb = [cst[:, FT + t:FT + t + 1] for t in range(FT)]
            bo_sb = [cst[:, 2 * FT + d:2 * FT + d + 1] for d in range(DT)]
            mk_sb = [cst[:, 2 * FT + DT + j:2 * FT + DT + j + 1]
                     for j in range(nt)]
            bv_sb = ptile([1, F], bf16, "bvrow")
            ones_sb = ptile([1, 128], bf16, "onesrow")
            zeros_sb = ptile([128, 512], bf16, "zeros")
            vscaf = ptile([128, GH * 128], bf16, "vscaf")
            qT_sb = [ptile([128, S], bf16, f"qT{t}") for t in range(FT)]
            kT_sb = [ptile([128, SK], bf16, f"kT{t}") for t in range(FT)]
            va_sb = [ptile([128, GH * 128], bf16, f"va{j}") for j in range(nt)]
            ot_sb = [ptile([128, S], bf16, f"ot{t}") for t in range(FT)]

            nc.vector.memset(zeros_sb[:], 0.0)
            nc.vector.memset(ones_sb[:], 1.0)

            def wqs(k, t):
                return wq_sb[:, k * F + t * 128:k * F + (t + 1) * 128]

            def wks(k, t):
                return wk_sb[:, k * F + t * 128:k * F + (t + 1) * 128]

            def wvs(k):
                return wv_sb[:, k * F:(k + 1) * F]

            def wos(t, do):
                return wo_sb[:, t * D + do * 128:t * D + (do + 1) * 128]

            # xq/xk SBUF layout is CHUNK-major: [chunk][k][cols] so each DMA
            # descriptor is one fully-contiguous [128, KT*w] block on both
            # the DRAM and SBUF side (descriptor issue time scales with the
            # number of contiguous runs, so fragmented patterns cost ~8us).
            def xqs(k, c, w=512):
                o = c * KT * 512
                return xq_sb[:, o + k * w:o + (k + 1) * w]

            XKO = [0]
            for (c0, c1) in KCH:
                XKO.append(XKO[-1] + KT * (c1 - c0))

            def xks(k, ci, w):
                o = XKO[ci]
                return xk_sb[:, o + k * w:o + (k + 1) * w]

            def xvs(st, k):
                return xv_sb[:, st * D + k * 128:st * D + (k + 1) * 128]

            # ---- DMA program. Per-queue sustained throughput is only
            # ~150GB/s (aggregate ~310), so the critical path (k chunk-a +
            # q i-half0, which gate the first exp units) is split in half-
            # descriptors and balanced across ALL THREE queues; later bytes
            # queue behind them. Every descriptor is a contiguous block.
            HB = KT // 2 * 512              # half of a 512-col chunk group
            # phase-1: sync gets wk + xka-hi + xq1-hi
            nc.sync.dma_start(out=wk_sb[:], in_=wk_d[:])
            nc.sync.dma_start(out=xk_sb[:, HB:2 * HB], in_=xk_d[0][:, HB:])
            nc.sync.dma_start(out=xq_sb[:, KT * 512 + HB:2 * KT * 512],
                              in_=xq_d[1][:, HB:])
            # scalar gets consts + wq + xq0 halves + xq1-lo
            nc.scalar.dma_start(out=cst[:], in_=cs_d[:])
            nc.scalar.dma_start(out=wq_sb[:], in_=wq_d[:])
            nc.scalar.dma_start(out=xq_sb[:, 0:HB], in_=xq_d[0][:, 0:HB])
            nc.scalar.dma_start(out=xq_sb[:, HB:KT * 512],
                                in_=xq_d[0][:, HB:])
            nc.scalar.dma_start(out=xq_sb[:, KT * 512:KT * 512 + HB],
                                in_=xq_d[1][:, 0:HB])
            # gpsimd gets the tiny constants + xka-lo
            nc.gpsimd.dma_start(out=vscaf[:], in_=vs_d[:])
            nc.gpsimd.dma_start(out=bv_sb[:], in_=bv_d[:].unsqueeze(0))
            nc.gpsimd.dma_start(out=xk_sb[:, 0:HB], in_=xk_d[0][:, 0:HB])

            # phase-2: k chunk-b first (it gates scores j4..7 and the
            # emission of proj_k_chunk(1) before h0), then v thirds
            # interleaved with the k tail chunk; q i-half1 behind on
            # scalar, v/o weights on gpsimd.
            nc.gpsimd.dma_start(out=wv_sb[:], in_=wv_d[:])
            xvs3 = []
            for c3 in range(NV3):
                s0, s1 = c3 * 3 * D, min((c3 * 3 + 3) * D, nt * D)
                xvs3.append((s0, s1))
            nc.sync.dma_start(out=xk_sb[:, XKO[1]:XKO[2]], in_=xk_d[1][:])
            nc.sync.dma_start(out=xv_sb[:, xvs3[0][0]:xvs3[0][1]],
                              in_=xv_d[:, xvs3[0][0]:xvs3[0][1]])
            for c in range(2, len(KCH)):
                nc.sync.dma_start(out=xk_sb[:, XKO[c]:XKO[c + 1]],
                                  in_=xk_d[c][:])
            for (s0, s1) in xvs3[1:]:
                nc.sync.dma_start(out=xv_sb[:, s0:s1], in_=xv_d[:, s0:s1])
            for c in range(2, NQC):
                nc.scalar.dma_start(
                    out=xq_sb[:, c * KT * 512:(c + 1) * KT * 512],
                    in_=xq_d[c][:])
            nc.gpsimd.dma_start(out=wo_sb[:], in_=wo_d[:])

            # va scaffold copies on DVE (idle until the first bias-add)
            for j in range(nt):
                nc.vector.tensor_copy(va_sb[j][:], vscaf[:])

            # PSUM: tag "pssc" = 3 rotating [128,1024] slots (6 banks) shared
            # by every transient accumulator (projections, scores, out-proj)
            # — 3 slots give the scores pipeline 2-deep lookahead so the exp
            # stream never waits. Tag "po" = 1 slot (2 banks) for the AV
            # accumulator; halves run serially in every head so only one po
            # is live at a time.
            with tc.tile_pool(name="psB", bufs=3, space="PSUM") as psB:

                def ps_tile(name, tag="pssc", bufs=3):
                    return psB.tile([128, 1024], mybir.dt.float32,
                                    name=name, tag=tag, bufs=bufs)

                # k projection, one 512-col chunk: gated only by its own xk
                # descriptor(s).
                def proj_k_chunk(ci):
                    c0, c1 = KCH[ci]
                    accs = [psB.tile([128, c1 - c0], mybir.dt.float32,
                                     name="kac", tag="pssc", bufs=3)
                            for t in range(FT)]
                    for k in range(KT):
                        for t in range(FT):
                            nc.tensor.matmul(
                                accs[t][:], lhsT=wks(k, t),
                                rhs=xks(k, ci, c1 - c0),
                                start=(k == 0), stop=(k == KT - 1))
                    for t in range(FT):
                        nc.vector.tensor_scalar_add(
                            kT_sb[t][:, c0:c1], accs[t][:], bk_sb[t])

                # q projection for one (f-tile, s-half): n-outer / k-inner so
                # chunk n is gated by xq descriptor sh*2+n only.
                def proj_q_half(t, sh):
                    acc = ps_tile("acc")
                    s0 = sh * 1024
                    for n in range(2):
                        for k in range(KT):
                            nc.tensor.matmul(
                                acc[:, n * 512:(n + 1) * 512],
                                lhsT=wqs(k, t),
                                rhs=xqs(k, sh * 2 + n),
                                start=(k == 0), stop=(k == KT - 1))
                    nc.vector.tensor_scalar_add(
                        qT_sb[t][:, s0:s0 + 1024], acc[:], bq_sb[t])

                # v projection for one seq tile (+bias via ones-row matmul).
                # va block per head: [ones col | zeros | v(64) at cols 64:128]
                # so po row 0 is the softmax denominator, rows 64:128 the
                # features (partition_broadcast only works from row 0).
                def vproj_unit(st):
                    pv = psB.tile([128, F], mybir.dt.float32,
                                  name="pv", tag="pssc", bufs=3)
                    for k in range(KT):
                        nc.tensor.matmul(
                            pv[:], lhsT=xvs(st, k),
                            rhs=wvs(k), start=(k == 0), stop=False)
                    nc.tensor.matmul(pv[:], lhsT=ones_sb[:], rhs=bv_sb[:],
                                     start=False, stop=True)
                    for h in range(GH):
                        d0 = h * 128 + HD
                        nc.vector.tensor_copy(
                            va_sb[st][:, d0:d0 + HD],
                            pv[:, h * HD:(h + 1) * HD])

                def scores_unit(h, half, j):
                    ht, off = h // 2, (h % 2) * HD
                    i0 = half * 1024
                    ps = ps_tile("pssc")
                    for n in range(2):
                        nc.tensor.matmul(
                            ps[:, n * 512:(n + 1) * 512],
                            lhsT=kT_sb[ht][off:off + HD,
                                           j * 128:(j + 1) * 128],
                            rhs=qT_sb[ht][off:off + HD,
                                          i0 + n * 512:i0 + (n + 1) * 512],
                            start=True, stop=True)
                    e = ep.tile([128, 1024], bf16, name="expT",
                                tag="expT", bufs=10)
                    nc.scalar.activation(e[:], ps[:], Exp,
                                         bias=mk_sb[j], scale=SCALE)
                    return e

                def av_unit(h, po_t, j, e):
                    for n in range(2):
                        nc.tensor.matmul(
                            po_t[:, n * 512:(n + 1) * 512],
                            lhsT=va_sb[j][:, h * 128:(h + 1) * 128],
                            rhs=e[:, n * 512:(n + 1) * 512],
                            start=(j == 0), stop=(j == nt - 1))

                # softmax divide: po row 0 is the denominator. Optionally one
                # DVE copy evacuates PSUM so the slot frees for the next
                # head; reciprocal + partition-broadcast + multiply. Odd
                # heads (ot rows 64:128) write ot in place; even heads need
                # the partition shift via a SBUF->SBUF DMA.
                def divide(h, half, po_t, use_pox):
                    ht = h // 2
                    i0 = half * 1024
                    if use_pox:
                        pox = dp.tile([128, 1024], f32, name="pox", tag="pox")
                        nc.vector.tensor_copy(pox[:], po_t[:])
                        src = pox
                    else:
                        src = po_t
                    rec = dp.tile([1, 1024], f32, name="rec", tag="rec")
                    nc.vector.reciprocal_approx_fast(out=rec[:],
                                                     in_=src[0:1, :])
                    # NB: broadcast src must be a separate tile and the dst
                    # must start at partition 0 — ucode constraints on HW.
                    recb = dp.tile([128, 1024], f32, name="recb", tag="recb")
                    nc.gpsimd.partition_broadcast(recb[:], rec[:])
                    if h % 2 == 1:
                        nc.vector.tensor_tensor(
                            out=ot_sb[ht][HD:128, i0:i0 + 1024],
                            in0=src[HD:128, :], in1=recb[HD:128, :],
                            op=mybir.AluOpType.mult)
                    else:
                        tmp = dp.tile([128, 1024], bf16, name="tmp", tag="tmp")
                        nc.vector.tensor_tensor(
                            out=tmp[HD:128, :],
                            in0=src[HD:128, :], in1=recb[HD:128, :],
                            op=mybir.AluOpType.mult)
                        nc.sync.dma_start(
                            out=ot_sb[ht][0:HD, i0:i0 + 1024],
                            in_=tmp[HD:128, :])

                # output projection unit: one do-tile of one i-half into the
                # group staging tile; ships a 1MB descriptor after each
                # 4-do group, alternating sync/gpsimd queues. use_scalar
                # picks the bias engine (ScalarE only when it is not
                # carrying the exp stream).
                ostate = {}

                def oproj_unit(ih, do, use_scalar):
                    i0 = ih * 1024
                    dg, dl = do // 4, do % 4
                    if dl == 0:
                        ostate[(ih, dg)] = osp.tile([128, 4096], bf16,
                                                    name="stg4", tag="stg")
                    stg = ostate[(ih, dg)]
                    pso = ps_tile("pso")
                    for n in range(2):
                        for t in range(FT):
                            nc.tensor.matmul(
                                pso[:, n * 512:(n + 1) * 512],
                                lhsT=wos(t, do),
                                rhs=ot_sb[t][:, i0 + n * 512:
                                             i0 + (n + 1) * 512],
                                start=(t == 0), stop=(t == FT - 1))
                    if use_scalar and do % 2 == 1:
                        nc.scalar.add(stg[:, dl * 1024:(dl + 1) * 1024],
                                      pso[:], bo_sb[do])
                    else:
                        nc.vector.tensor_scalar_add(
                            stg[:, dl * 1024:(dl + 1) * 1024],
                            pso[:], bo_sb[do])
                    if dl == 3:
                        eng = nc.sync if dg == 0 else nc.gpsimd
                        eng.dma_start(
                            out=out_d[:, dg * 4:(dg + 1) * 4, i0:i0 + 1024],
                            in_=stg[:].rearrange("p (d i) -> p d i", d=4))

                # ---------------- emission schedule ----------------
                # chunk-a of k and the q i-half0 projection gate the first
                # exp; k chunk-b has later-arriving data and is emitted
                # after so it can't block the first scores; the small k
                # tail chunk is folded into h0-half0 (kT j-tile 8 is first
                # read by scores j8).
                proj_k_chunk(0)
                proj_q_half(0, 0)
                proj_k_chunk(1)

                # every head runs its two i-halves serially (one exp unit
                # per j). Folds fill the exp-wait bubbles with real work:
                # h0-half0 carries the v-proj units (lagged one unit behind
                # their AV consumer so the late xv stream can't stall the
                # scores/exp pipeline) and the q t0-half1 burst near the
                # end (xq chunks 2,3 arrive mid-loop); h1 carries the q t1
                # bursts.
                for h in range(GH):
                    for half in range(2):
                        po_t = psB.tile([128, 1024], mybir.dt.float32,
                                        name="po", tag="po", bufs=1)
                        if h == 0 and half == 0:
                            es = []
                            for j in range(nt):
                                es.append(scores_unit(0, 0, j))
                                if j == 4 and len(KCH) > 2:
                                    proj_k_chunk(2)
                                if j == max(nt - 3, 1):
                                    proj_q_half(0, 1)
                                if j >= 1:
                                    vproj_unit(j - 1)
                                    av_unit(0, po_t, j - 1, es[j - 1])
                            vproj_unit(nt - 1)
                            av_unit(0, po_t, nt - 1, es[nt - 1])
                            e = None
                        else:
                            for j in range(nt):
                                e = scores_unit(h, half, j)
                                if h == 1 and j == min(1, nt - 1):
                                    proj_q_half(1, half)
                                av_unit(h, po_t, j, e)
                        last = (h == GH - 1 and half == 1)
                        divide(h, half, po_t, use_pox=not last)
                        if last:
                            # ot half0 completed back at divide(h3,0), so
                            # out_proj(0) streams immediately after the last
                            # AV while the half1 divide chain runs on DVE/
                            # GpSimd; out_proj(1) follows as ot half1 lands.
                            for do in range(DT):
                                oproj_unit(0, do, use_scalar=True)
                            for do in range(DT):
                                oproj_unit(1, do, use_scalar=True)

    nc.compile()
    return nc


def kernel(query, key, value, src_mask, Wq, bq, Wk, bk, Wv, bv, Wo, bo, nhead):
    global LAST_EXEC_NS, LAST_RESULTS
    import ml_dtypes
    from concourse.bass_utils import run_bass_kernel_spmd

    assert int(nhead) == H
    bf16 = ml_dtypes.bfloat16
    query = np.asarray(query, dtype=np.float32)
    key = np.asarray(key, dtype=np.float32)
    value = np.asarray(value, dtype=np.float32)
    src_mask = np.asarray(src_mask)
    Wq, bq = np.asarray(Wq, np.float32), np.asarray(bq, np.float32)
    Wk, bk = np.asarray(Wk, np.float32), np.asarray(bk, np.float32)
    Wv, bv = np.asarray(Wv, np.float32), np.asarray(bv, np.float32)
    Wo, bo = np.asarray(Wo, np.float32), np.asarray(bo, np.float32)

    # gather unmasked key/value positions (masked keys contribute exactly 0)
    idxs = [np.flatnonzero(~src_mask[b]) for b in range(B)]
    nt = max(1, (max(len(ix) for ix in idxs) + 127) // 128)
    SK = nt * 128
    KCH = _chunks(SK)

    if nt not in _STATE:
        _STATE[nt] = _build(nt)
    nc = _STATE[nt]

    def tile_p(mat2d):
        # [KT*128, W] -> [128, KT, W] (partition-major k-tiling)
        w = mat2d.shape[1]
        return np.ascontiguousarray(
            mat2d.reshape(KT, 128, w).transpose(1, 0, 2))

    xq_c, xk_c, xv2, maskf = [], [], [], []
    for b in range(B):
        qt = tile_p(query[b].T.astype(bf16))      # [128, KT, S]
        xq_c.append([np.ascontiguousarray(
            qt[:, :, c * 512:(c + 1) * 512]).reshape(128, -1)
            for c in range(NQC)])
        ix = idxs[b]
        nu = len(ix)
        kg = np.zeros((SK, D), np.float32)
        kg[:nu] = key[b][ix]
        kt = tile_p(kg.T.astype(bf16))            # [128, KT, SK]
        xk_c.append([np.ascontiguousarray(kt[:, :, c0:c1]).reshape(128, -1)
                     for (c0, c1) in KCH])
        vg = np.zeros((SK, D), np.float32)
        vg[:nu] = value[b][ix]
        # xv2[p, st*D + k*128+c] = vg.T[k*128+p, st*128+c]
        xv2.append(np.ascontiguousarray(
            vg.T.reshape(KT, 128, nt, 128).transpose(1, 2, 0, 3)
            .reshape(128, nt * D)).astype(bf16))
        mk = np.where(np.arange(SK) < nu, np.float32(0), NEG)
        maskf.append(np.ascontiguousarray(
            mk.reshape(nt, 128).T.astype(np.float32)))

    # va scaffold: ones column at the head block start (denominator row 0)
    vscaf = np.zeros((128, GH * 128), np.float32)
    for h in range(GH):
        vscaf[:, h * 128] = 1.0
    vscaf = vscaf.astype(bf16)

    wq2, wk2, wv2, wo2, cst, bvs = [], [], [], [], [], []
    for g in range(NCORES // B):
        gs, ge = g * F, (g + 1) * F
        wq2.append(np.ascontiguousarray(
            tile_p(Wq[gs:ge, :].T.astype(bf16)).reshape(128, KT * F)))
        wk2.append(np.ascontiguousarray(
            tile_p(Wk[gs:ge, :].T.astype(bf16)).reshape(128, KT * F)))
        wv2.append(np.ascontiguousarray(
            tile_p(Wv[gs:ge, :].T.astype(bf16)).reshape(128, KT * F)))
        # wo2[p, t*D+c] = Wo[:, gs:ge].T[t*128+p, c]
        woT = Wo[:, gs:ge].T.astype(bf16)          # [F, D]
        wo2.append(np.ascontiguousarray(
            woT.reshape(FT, 128, D).transpose(1, 0, 2).reshape(128, FT * D)))
        bq2 = bq[gs:ge].reshape(FT, 128).T
        bk2 = bk[gs:ge].reshape(FT, 128).T
        bvs.append(bv[gs:ge].astype(bf16))
        bo2 = bo.reshape(DT, 128).T if g == 0 else np.zeros((128, DT),
                                                            np.float32)
        cst.append((bq2, bk2, bo2))

    in_maps = []
    for c in range(NCORES):
        b, g = c // (NCORES // B), c % (NCORES // B)
        bq2, bk2, bo2 = cst[g]
        cpack = np.ascontiguousarray(np.concatenate(
            [bq2, bk2, bo2, maskf[b]], axis=1).astype(np.float32))
        m = {"xv2": xv2[b], "wq2": wq2[g], "wk2": wk2[g], "wv2": wv2[g],
             "wo2": wo2[g], "cst": cpack, "bv": bvs[g], "vscaf": vscaf}
        for ci in range(NQC):
            m[f"xq{ci}"] = xq_c[b][ci]
        for ci in range(len(KCH)):
            m[f"xk{ci}"] = xk_c[b][ci]
        in_maps.append(m)

    kwargs = {}
    if TRACE:
        kwargs = dict(trace=True)
    res = run_bass_kernel_spmd(nc, in_maps, core_ids=list(range(NCORES)),
                               **kwargs)
    LAST_EXEC_NS = res.exec_time_ns
    LAST_RESULTS = res

    out = np.empty((B, S, D), dtype=np.float32)
    for b in range(B):
        acc = res.results[b * (NCORES // B)]["out2"].astype(np.float32)
        for g in range(1, NCORES // B):
            acc = acc + res.results[b * (NCORES // B) + g]["out2"]
        # out2 [128, DT, S] -> [D, S] -> [S, D]
        out[b] = acc.transpose(1, 0, 2).reshape(D, S).T
    return out


# revision 58
# speedup vs baseline: 1.1343x; 1.0047x over previous
"""Multihead attention (B=2, S=2048, D=1024, H=16) on 8 TRN2 NeuronCores.

Sharding: core c -> batch b = c//4, head-group g = c%4 (4 heads, 256 features).
Each core computes q/k/v projections for its 256 features, attention for its
4 heads, and a row-parallel partial of the output projection. Host sums the
4 partials per batch (row-parallel TP unshard) and transposes back.

Mask gather: src_mask is per key position and ~half the keys are masked
(exp underflows to exactly 0), so the host gathers only unmasked key/value
positions, padded to NT*128 (NT=9 for ~1024 survivors). Scores, exp, AV,
k-proj and v-proj all shrink by NT/16. Padding columns get a -9e9 exp bias
so they contribute exactly 0, like masked keys did.

The run is ScalarE-bound in the middle: softmax exp is 72 ACTIVATE ops of
[128,1024] at ~1.1us each (~80us floor). Everything else is scheduled to
keep that stream fed:

DMA: all inputs are host-pre-tiled so each tensor is ONE descriptor
(descriptor issue is ~0.6us each and was the old bottleneck). Three queues
(Sync / Scalar / GpSimd), each with critical phase-1 bytes first and
phase-2 bytes queued behind them on the same queue:
  sync:   wk, xk chunks (512-col groups, all k-tiles)    | xv thirds
  scalar: consts, wq, xq i-chunks 0,1 (i 0:1024)         | xq chunks 2,3
  gpsimd: vscaf, bv, wv                                  | wo
First scores need only k/q-path phase-1 (~4.3MB) -> exp starts ~13us.

Per-core pipeline (all matmuls bf16 with f32 PSUM accumulation):
  k-proj is chunk-outer/k-inner so each 512-col chunk finishes as its xk
  descriptor lands (per-chunk bias-add frees scores j-tiles 0..3 early).
  q-proj t0 half0 before h0; q t0 half1 folded into h0-half0's j-loop
  (xq chunks 2,3 arrive mid-h0); q t1 folded into h1's j-loop. v-proj
  units folded just-in-time into h0-half0 (vproj(j) right before AV(j)).
  h0 runs its two i-halves serially (half1 depends on late xq bytes);
  h1..h3 interleave halves per j as before. Attention per head, per
  j-tile: scoresT [128 j, 1024 i] in PSUM, exp(scale*x + mask_j) fused on
  ScalarE (mask = per-partition bias), then AV with va stationary:
  po += va_j^T @ expT_j. Each head's va block is [ones | zeros | v(64)]
  so po row 0 is the softmax denominator and rows 64:128 the features.
  The divide is a DVE reciprocal, a GpSimd partition-broadcast (dst must
  start at partition 0 and src must be a separate tile), and a DVE
  multiply: odd heads write ot rows 64:128 in place, even heads go
  through a bf16 staging tile + SBUF->SBUF DMA for the partition shift.

Output projection accumulates 4 do-tiles into one [128, 4096] staging
tile and ships it as ONE 1MB descriptor, alternating sync/gpsimd queues
so the 4MB output stream overlaps the remaining compute.
"""

import math

import numpy as np

B, S, D, H = 2, 2048, 1024, 16
NCORES = 8
GH = 4                  # heads per core
HD = D // H             # 64
F = GH * HD             # 256 local features
SCALE = 1.0 / math.sqrt(HD)
NEG = np.float32(-9e9)

KT = D // 128           # 8 contraction tiles (projections)
FT = F // 128           # 2 local-feature tiles
DT = D // 128           # 8 output-feature tiles
NQC = S // 512          # 4 xq i-chunks

TRACE = False           # set by test harness; requires antenv.axon_hooks wired
LAST_EXEC_NS = None
LAST_RESULTS = None

_STATE = {}


def _chunks(width):
    c, out = 0, []
    while c < width:
        out.append((c, min(c + 512, width)))
        c += 512
    return out


def _build(nt):
    import concourse.bacc as bacc
    import concourse.mybir as mybir
    from concourse.tile import TileContext

    f32 = mybir.dt.float32
    bf16 = mybir.dt.bfloat16
    Exp = mybir.ActivationFunctionType.Exp

    SK = nt * 128               # gathered key/value length
    KCH = _chunks(SK)           # k-proj column chunks
    NV3 = (nt + 2) // 3         # xv thirds

    nc = bacc.Bacc("TRN2", target_bir_lowering=False, debug=False,
                   num_devices=NCORES)

    # host-pre-tiled inputs, one DMA descriptor each
    xq_d = [nc.declare_dram_parameter(f"xq{c}", [128, KT * 512], bf16,
                                      isOutput=False) for c in range(NQC)]
    xk_d = [nc.declare_dram_parameter(f"xk{c}", [128, KT * (c1 - c0)], bf16,
                                      isOutput=False)
            for c, (c0, c1) in enumerate(KCH)]
    xv_d = nc.declare_dram_parameter("xv2", [128, nt * D], bf16,
                                     isOutput=False)
    wq_d = nc.declare_dram_parameter("wq2", [128, KT * F], bf16, isOutput=False)
    wk_d = nc.declare_dram_parameter("wk2", [128, KT * F], bf16, isOutput=False)
    wv_d = nc.declare_dram_parameter("wv2", [128, KT * F], bf16, isOutput=False)
    wo_d = nc.declare_dram_parameter("wo2", [128, FT * D], bf16, isOutput=False)
    # packed per-partition constants: [bq(FT) | bk(FT) | bo(DT) | mask(nt)]
    cs_d = nc.declare_dram_parameter("cst", [128, FT + FT + DT + nt], f32,
                                     isOutput=False)
    bv_d = nc.declare_dram_parameter("bv", [F], bf16, isOutput=False)
    # va scaffold: zeros with a ones column per head at its parity slot
    vs_d = nc.declare_dram_parameter("vscaf", [128, GH * 128], bf16,
                                     isOutput=False)
    out_d = nc.declare_dram_parameter("out2", [128, DT, S], bf16,
                                      isOutput=True)

    with TileContext(nc) as tc:
        with tc.tile_pool(name="persist", bufs=1) as pp, \
             tc.tile_pool(name="expp", bufs=10) as ep, \
             tc.tile_pool(name="ostage", bufs=2) as osp, \
             tc.tile_pool(name="divp", bufs=2) as dp:

            def ptile(shape, dtype, name):
                return pp.tile(shape, dtype, name=name, tag=name)

            # ---- persistent SBUF tensors ----
            wq_sb = ptile([128, KT * F], bf16, "wq")
            wk_sb = ptile([128, KT * F], bf16, "wk")
            wv_sb = ptile([128, KT * F], bf16, "wv")
            wo_sb = ptile([128, FT * D], bf16, "wo")
            xq_sb = ptile([128, KT * S], bf16, "xq")
            xk_sb = ptile([128, KT * SK], bf16, "xk")
            xv_sb = ptile([128, nt * D], bf16, "xv")
            cst = ptile([128, FT + FT + DT + nt], f32, "cst")
            bq_sb = [cst[:, t:t + 1] for t in range(FT)]
            bk_sb = [cst[:, FT + t:FT + t + 1] for t in range(FT)]
            bo_sb = [cst[:, 2 * FT + d:2 * FT + d + 1] for d in range(DT)]
            mk_sb = [cst[:, 2 * FT + DT + j:2 * FT + DT + j + 1]
                     for j in range(nt)]
            bv_sb = ptile([1, F], bf16, "bvrow")
            ones_sb = ptile([1, 128], bf16, "onesrow")
            zeros_sb = ptile([128, 512], bf16, "zeros")
            vscaf = ptile([128, GH * 128], bf16, "vscaf")
            qT_sb = [ptile([128, S], bf16, f"qT{t}") for t in range(FT)]
            kT_sb = [ptile([128, SK], bf16, f"kT{t}") for t in range(FT)]
            va_sb = [ptile([128, GH * 128], bf16, f"va{j}") for j in range(nt)]
            ot_sb = [ptile([128, S], bf16, f"ot{t}") for t in range(FT)]

            nc.vector.memset(zeros_sb[:], 0.0)
            nc.vector.memset(ones_sb[:], 1.0)

            def wqs(k, t):
                return wq_sb[:, k * F + t * 128:k * F + (t + 1) * 128]

            def wks(k, t):
                return wk_sb[:, k * F + t * 128:k * F + (t + 1) * 128]

            def wvs(k):
                return wv_sb[:, k * F:(k + 1) * F]

            def wos(t, do):
                return wo_sb[:, t * D + do * 128:t * D + (do + 1) * 128]

            # xq/xk SBUF layout is CHUNK-major: [chunk][k][cols] so each DMA
            # descriptor is one fully-contiguous [128, KT*w] block on both
            # the DRAM and SBUF side (descriptor issue time scales with the
            # number of contiguous runs, so fragmented patterns cost ~8us).
            def xqs(k, c, w=512):
                o = c * KT * 512
                return xq_sb[:, o + k * w:o + (k + 1) * w]

            XKO = [0]
            for (c0, c1) in KCH:
                XKO.append(XKO[-1] + KT * (c1 - c0))

            def xks(k, ci, w):
                o = XKO[ci]
                return xk_sb[:, o + k * w:o + (k + 1) * w]

            def xvs(st, k):
                return xv_sb[:, st * D + k * 128:st * D + (k + 1) * 128]

            # ---- DMA program. Per-queue sustained throughput is only
            # ~150GB/s (aggregate ~310), so the critical path (k chunk-a +
            # q i-half0, which gate the first exp units) is split in half-
            # descriptors and balanced across ALL THREE queues; later bytes
            # queue behind them. Every descriptor is a contiguous block.
            HB = KT // 2 * 512              # half of a 512-col chunk group
            # phase-1 (~1.1MB per queue, balanced so all three finish
            # together): what the first exp units need — k chunk-a, q
            # i-half0 chunks, weights, consts.
            nc.sync.dma_start(out=wk_sb[:], in_=wk_d[:])
            nc.sync.dma_start(out=xq_sb[:, KT * 512:KT * 512 + HB],
                              in_=xq_d[1][:, 0:HB])
            nc.sync.dma_start(out=xq_sb[:, KT * 512 + HB:2 * KT * 512],
                              in_=xq_d[1][:, HB:])
            nc.scalar.dma_start(out=cst[:], in_=cs_d[:])
            nc.scalar.dma_start(out=wq_sb[:], in_=wq_d[:])
            nc.scalar.dma_start(out=xq_sb[:, 0:HB], in_=xq_d[0][:, 0:HB])
            nc.scalar.dma_start(out=xq_sb[:, HB:KT * 512],
                                in_=xq_d[0][:, HB:])
            nc.gpsimd.dma_start(out=vscaf[:], in_=vs_d[:])
            nc.gpsimd.dma_start(out=bv_sb[:], in_=bv_d[:].unsqueeze(0))
            nc.gpsimd.dma_start(out=xk_sb[:, 0:HB], in_=xk_d[0][:, 0:HB])
            nc.gpsimd.dma_start(out=xk_sb[:, HB:2 * HB], in_=xk_d[0][:, HB:])

            # phase-2, queued behind phase-1 per queue: k chunk-b (gates
            # scores j4..7, folded into h0-half0), v thirds just-in-time
            # for the vproj folds, q i-half1, k tail, v/o weights.
            xvs3 = []
            for c3 in range(NV3):
                s0, s1 = c3 * 3 * D, min((c3 * 3 + 3) * D, nt * D)
                xvs3.append((s0, s1))
            nc.sync.dma_start(out=xk_sb[:, XKO[1]:XKO[2]], in_=xk_d[1][:])
            for (s0, s1) in xvs3:
                nc.sync.dma_start(out=xv_sb[:, s0:s1], in_=xv_d[:, s0:s1])
            nc.gpsimd.dma_start(out=wv_sb[:], in_=wv_d[:])
            for c in range(2, NQC):
                nc.scalar.dma_start(
                    out=xq_sb[:, c * KT * 512:(c + 1) * KT * 512],
                    in_=xq_d[c][:])
            for c in range(2, len(KCH)):
                nc.scalar.dma_start(out=xk_sb[:, XKO[c]:XKO[c + 1]],
                                    in_=xk_d[c][:])
            nc.gpsimd.dma_start(out=wo_sb[:], in_=wo_d[:])

            # va scaffold copies on DVE (idle until the first bias-add)
            for j in range(nt):
                nc.vector.tensor_copy(va_sb[j][:], vscaf[:])

            # PSUM: tag "pssc" = 3 rotating [128,1024] slots (6 banks) shared
            # by every transient accumulator (projections, scores, out-proj)
            # — 3 slots give the scores pipeline 2-deep lookahead so the exp
            # stream never waits. Tag "po" = 1 slot (2 banks) for the AV
            # accumulator; halves run serially in every head so only one po
            # is live at a time.
            with tc.tile_pool(name="psB", bufs=3, space="PSUM") as psB:

                def ps_tile(name, tag="pssc", bufs=3):
                    return psB.tile([128, 1024], mybir.dt.float32,
                                    name=name, tag=tag, bufs=bufs)

                # small keep-warm bursts, sized BELOW the expected data
                # gaps: the PE clock (0.65/1.2/2.4GHz) ramps only while
                # continuously busy, and the DMA-gated head otherwise runs
                # every projection burst cold. Oversized bursts would delay
                # real work (measured), so n stays tiny.
                def emit_warms(n):
                    w = ps_tile("warm")
                    for i in range(n):
                        nc.tensor.matmul(
                            w[:, (i % 2) * 512:(i % 2) * 512 + 512],
                            lhsT=zeros_sb[:, 0:128], rhs=zeros_sb[:],
                            start=True, stop=True)

                # k projection, one 512-col chunk: gated only by its own xk
                # descriptor(s).
                def proj_k_chunk(ci):
                    c0, c1 = KCH[ci]
                    accs = [psB.tile([128, c1 - c0], mybir.dt.float32,
                                     name="kac", tag="pssc", bufs=3)
                            for t in range(FT)]
                    for k in range(KT):
                        if ci == 0 and k == KT // 2:
                            emit_warms(2)
                        for t in range(FT):
                            nc.tensor.matmul(
                                accs[t][:], lhsT=wks(k, t),
                                rhs=xks(k, ci, c1 - c0),
                                start=(k == 0), stop=(k == KT - 1))
                    for t in range(FT):
                        nc.vector.tensor_scalar_add(
                            kT_sb[t][:, c0:c1], accs[t][:], bk_sb[t])

                # q projection for one (f-tile, s-half): n-outer / k-inner so
                # chunk n is gated by xq descriptor sh*2+n only.
                def proj_q_half(t, sh, warmup=False):
                    acc = ps_tile("acc")
                    s0 = sh * 1024
                    for n in range(2):
                        if warmup:
                            emit_warms(2)
                        for k in range(KT):
                            nc.tensor.matmul(
                                acc[:, n * 512:(n + 1) * 512],
                                lhsT=wqs(k, t),
                                rhs=xqs(k, sh * 2 + n),
                                start=(k == 0), stop=(k == KT - 1))
                    nc.vector.tensor_scalar_add(
                        qT_sb[t][:, s0:s0 + 1024], acc[:], bq_sb[t])

                # v projection for one seq tile (+bias via ones-row matmul).
                # va block per head: [ones col | zeros | v(64) at cols 64:128]
                # so po row 0 is the softmax denominator, rows 64:128 the
                # features (partition_broadcast only works from row 0).
                def vproj_unit(st):
                    pv = psB.tile([128, F], mybir.dt.float32,
                                  name="pv", tag="pssc", bufs=3)
                    for k in range(KT):
                        nc.tensor.matmul(
                            pv[:], lhsT=xvs(st, k),
                            rhs=wvs(k), start=(k == 0), stop=False)
                    nc.tensor.matmul(pv[:], lhsT=ones_sb[:], rhs=bv_sb[:],
                                     start=False, stop=True)
                    for h in range(GH):
                        d0 = h * 128 + HD
                        nc.vector.tensor_copy(
                            va_sb[st][:, d0:d0 + HD],
                            pv[:, h * HD:(h + 1) * HD])

                def scores_unit(h, half, j):
                    ht, off = h // 2, (h % 2) * HD
                    i0 = half * 1024
                    ps = ps_tile("pssc")
                    for n in range(2):
                        nc.tensor.matmul(
                            ps[:, n * 512:(n + 1) * 512],
                            lhsT=kT_sb[ht][off:off + HD,
                                           j * 128:(j + 1) * 128],
                            rhs=qT_sb[ht][off:off + HD,
                                          i0 + n * 512:i0 + (n + 1) * 512],
                            start=True, stop=True)
                    e = ep.tile([128, 1024], bf16, name="expT",
                                tag="expT", bufs=10)
                    nc.scalar.activation(e[:], ps[:], Exp,
                                         bias=mk_sb[j], scale=SCALE)
                    return e

                def av_unit(h, po_t, j, e):
                    for n in range(2):
                        nc.tensor.matmul(
                            po_t[:, n * 512:(n + 1) * 512],
                            lhsT=va_sb[j][:, h * 128:(h + 1) * 128],
                            rhs=e[:, n * 512:(n + 1) * 512],
                            start=(j == 0), stop=(j == nt - 1))

                # softmax divide: po row 0 is the denominator. Optionally one
                # DVE copy evacuates PSUM so the slot frees for the next
                # head; reciprocal + partition-broadcast + multiply. Odd
                # heads (ot rows 64:128) write ot in place; even heads need
                # the partition shift via a SBUF->SBUF DMA.
                def divide(h, half, po_t, use_pox):
                    ht = h // 2
                    i0 = half * 1024
                    if use_pox:
                        pox = dp.tile([128, 1024], f32, name="pox", tag="pox")
                        nc.vector.tensor_copy(pox[:], po_t[:])
                        src = pox
                    else:
                        src = po_t
                    rec = dp.tile([1, 1024], f32, name="rec", tag="rec")
                    nc.vector.reciprocal_approx_fast(out=rec[:],
                                                     in_=src[0:1, :])
                    # NB: broadcast src must be a separate tile and the dst
                    # must start at partition 0 — ucode constraints on HW.
                    recb = dp.tile([128, 1024], f32, name="recb", tag="recb")
                    nc.gpsimd.partition_broadcast(recb[:], rec[:])
                    if h % 2 == 1:
                        nc.vector.tensor_tensor(
                            out=ot_sb[ht][HD:128, i0:i0 + 1024],
                            in0=src[HD:128, :], in1=recb[HD:128, :],
                            op=mybir.AluOpType.mult)
                    else:
                        tmp = dp.tile([128, 1024], bf16, name="tmp", tag="tmp")
                        nc.vector.tensor_tensor(
                            out=tmp[HD:128, :],
                            in0=src[HD:128, :], in1=recb[HD:128, :],
                            op=mybir.AluOpType.mult)
                        nc.sync.dma_start(
                            out=ot_sb[ht][0:HD, i0:i0 + 1024],
                            in_=tmp[HD:128, :])

                # output projection unit: one do-tile of one i-half into the
                # group staging tile; ships a 1MB descriptor after each
                # 4-do group, alternating sync/gpsimd queues. use_scalar
                # picks the bias engine (ScalarE only when it is not
                # carrying the exp stream).
                ostate = {}

                def oproj_unit(ih, do, use_scalar):
                    i0 = ih * 1024
                    dg, dl = do // 2, do % 2
                    if dl == 0:
                        ostate[(ih, dg)] = osp.tile([128, 2048], bf16,
                                                    name="stg2", tag="stg",
                                                    bufs=3)
                    stg = ostate[(ih, dg)]
                    pso = ps_tile("pso")
                    for n in range(2):
                        for t in range(FT):
                            nc.tensor.matmul(
                                pso[:, n * 512:(n + 1) * 512],
                                lhsT=wos(t, do),
                                rhs=ot_sb[t][:, i0 + n * 512:
                                             i0 + (n + 1) * 512],
                                start=(t == 0), stop=(t == FT - 1))
                    if use_scalar and do % 2 == 1:
                        nc.scalar.add(stg[:, dl * 1024:(dl + 1) * 1024],
                                      pso[:], bo_sb[do])
                    else:
                        nc.vector.tensor_scalar_add(
                            stg[:, dl * 1024:(dl + 1) * 1024],
                            pso[:], bo_sb[do])
                    if dl == 1:
                        eng = nc.sync if dg % 2 == 0 else nc.gpsimd
                        eng.dma_start(
                            out=out_d[:, dg * 2:(dg + 1) * 2, i0:i0 + 1024],
                            in_=stg[:].rearrange("p (d i) -> p d i", d=2))

                # ---------------- emission schedule ----------------
                # chunk-a of k and the q i-half0 projection gate the first
                # exp; k chunks b/c have later-arriving data and are folded
                # into h0-half0 (kT j-tiles 4..7 are first read by scores
                # j4, j-tile 8 by scores j8).
                emit_warms(2)
                proj_k_chunk(0)
                proj_q_half(0, 0, warmup=True)

                # every head runs its two i-halves serially (one exp unit
                # per j). Folds fill the exp-wait bubbles with real work:
                # h0-half0 carries the v-proj units (lagged one unit behind
                # their AV consumer so the late xv stream can't stall the
                # scores/exp pipeline) and the q t0-half1 burst near the
                # end (xq chunks 2,3 arrive mid-loop); h1 carries the q t1
                # bursts.
                for h in range(GH):
                    for half in range(2):
                        po_t = psB.tile([128, 1024], mybir.dt.float32,
                                        name="po", tag="po", bufs=1)
                        if h == 0 and half == 0:
                            es = []
                            for j in range(nt):
                                es.append(scores_unit(0, 0, j))
                                if j == 1:
                                    proj_k_chunk(1)
                                if j == 4 and len(KCH) > 2:
                                    proj_k_chunk(2)
                                if j == max(nt - 3, 1):
                                    proj_q_half(0, 1)
                                if j >= 1:
                                    vproj_unit(j - 1)
                                    av_unit(0, po_t, j - 1, es[j - 1])
                            vproj_unit(nt - 1)
                            av_unit(0, po_t, nt - 1, es[nt - 1])
                        elif h == GH - 1 and half == 1:
                            # ot half0 completes at divide(h3,0) a few units
                            # in: fold most of the i-half0 output projection
                            # here (bias on DVE; ScalarE carries the exps).
                            for j in range(nt):
                                e = scores_unit(h, half, j)
                                if 3 <= j < min(nt, 9):
                                    oproj_unit(0, j - 3, use_scalar=False)
                                av_unit(h, po_t, j, e)
                        else:
                            for j in range(nt):
                                e = scores_unit(h, half, j)
                                if h == 1 and j == min(1, nt - 1):
                                    proj_q_half(1, half)
                                av_unit(h, po_t, j, e)
                        last = (h == GH - 1 and half == 1)
                        divide(h, half, po_t, use_pox=not last)
                        if last:
                            # leftover i-half0 do-tiles keep the PE busy
                            # while the half1 divide chain runs on DVE/
                            # GpSimd; out_proj(1) starts as ot half1 lands.
                            ndone = max(0, min(nt, 9) - 3)
                            for do in range(ndone, DT):
                                oproj_unit(0, do, use_scalar=True)
                            for do in range(DT):
                                oproj_unit(1, do, use_scalar=True)

    nc.compile()
    return nc


def kernel(query, key, value, src_mask, Wq, bq, Wk, bk, Wv, bv, Wo, bo, nhead):
    global LAST_EXEC_NS, LAST_RESULTS
    import ml_dtypes
    from concourse.bass_utils import run_bass_kernel_spmd

    assert int(nhead) == H
    bf16 = ml_dtypes.bfloat16
    query = np.asarray(query, dtype=np.float32)
    key = np.asarray(key, dtype=np.float32)
    value = np.asarray(value, dtype=np.float32)
    src_mask = np.asarray(src_mask)
    Wq, bq = np.asarray(Wq, np.float32), np.asarray(bq, np.float32)
    Wk, bk = np.asarray(Wk, np.float32), np.asarray(bk, np.float32)
    Wv, bv = np.asarray(Wv, np.float32), np.asarray(bv, np.float32)
    Wo, bo = np.asarray(Wo, np.float32), np.asarray(bo, np.float32)

    # gather unmasked key/value positions (masked keys contribute exactly 0)
    idxs = [np.flatnonzero(~src_mask[b]) for b in range(B)]
    nt = max(1, (max(len(ix) for ix in idxs) + 127) // 128)
    SK = nt * 128
    KCH = _chunks(SK)

    if nt not in _STATE:
        _STATE[nt] = _build(nt)
    nc = _STATE[nt]

    def tile_p(mat2d):
        # [KT*128, W] -> [128, KT, W] (partition-major k-tiling)
        w = mat2d.shape[1]
        return np.ascontiguousarray(
            mat2d.reshape(KT, 128, w).transpose(1, 0, 2))

    xq_c, xk_c, xv2, maskf = [], [], [], []
    for b in range(B):
        qt = tile_p(query[b].T.astype(bf16))      # [128, KT, S]
        xq_c.append([np.ascontiguousarray(
            qt[:, :, c * 512:(c + 1) * 512]).reshape(128, -1)
            for c in range(NQC)])
        ix = idxs[b]
        nu = len(ix)
        kg = np.zeros((SK, D), np.float32)
        kg[:nu] = key[b][ix]
        kt = tile_p(kg.T.astype(bf16))            # [128, KT, SK]
        xk_c.append([np.ascontiguousarray(kt[:, :, c0:c1]).reshape(128, -1)
                     for (c0, c1) in KCH])
        vg = np.zeros((SK, D), np.float32)
        vg[:nu] = value[b][ix]
        # xv2[p, st*D + k*128+c] = vg.T[k*128+p, st*128+c]
        xv2.append(np.ascontiguousarray(
            vg.T.reshape(KT, 128, nt, 128).transpose(1, 2, 0, 3)
            .reshape(128, nt * D)).astype(bf16))
        mk = np.where(np.arange(SK) < nu, np.float32(0), NEG)
        maskf.append(np.ascontiguousarray(
            mk.reshape(nt, 128).T.astype(np.float32)))

    # va scaffold: ones column at the head block start (denominator row 0)
    vscaf = np.zeros((128, GH * 128), np.float32)
    for h in range(GH):
        vscaf[:, h * 128] = 1.0
    vscaf = vscaf.astype(bf16)

    wq2, wk2, wv2, wo2, cst, bvs = [], [], [], [], [], []
    for g in range(NCORES // B):
        gs, ge = g * F, (g + 1) * F
        wq2.append(np.ascontiguousarray(
            tile_p(Wq[gs:ge, :].T.astype(bf16)).reshape(128, KT * F)))
        wk2.append(np.ascontiguousarray(
            tile_p(Wk[gs:ge, :].T.astype(bf16)).reshape(128, KT * F)))
        wv2.append(np.ascontiguousarray(
            tile_p(Wv[gs:ge, :].T.astype(bf16)).reshape(128, KT * F)))
        # wo2[p, t*D+c] = Wo[:, gs:ge].T[t*128+p, c]
        woT = Wo[:, gs:ge].T.astype(bf16)          # [F, D]
        wo2.append(np.ascontiguousarray(
            woT.reshape(FT, 128, D).transpose(1, 0, 2).reshape(128, FT * D)))
        bq2 = bq[gs:ge].reshape(FT, 128).T
        bk2 = bk[gs:ge].reshape(FT, 128).T
        bvs.append(bv[gs:ge].astype(bf16))
        bo2 = bo.reshape(DT, 128).T if g == 0 else np.zeros((128, DT),
                                                            np.float32)
        cst.append((bq2, bk2, bo2))

    in_maps = []
    for c in range(NCORES):
        b, g = c // (NCORES // B), c % (NCORES // B)
        bq2, bk2, bo2 = cst[g]
        cpack = np.ascontiguousarray(np.concatenate(
            [bq2, bk2, bo2, maskf[b]], axis=1).astype(np.float32))
        m = {"xv2": xv2[b], "wq2": wq2[g], "wk2": wk2[g], "wv2": wv2[g],
             "wo2": wo2[g], "cst": cpack, "bv": bvs[g], "vscaf": vscaf}
        for ci in range(NQC):
            m[f"xq{ci}"] = xq_c[b][ci]
        for ci in range(len(KCH)):
            m[f"xk{ci}"] = xk_c[b][ci]
        in_maps.append(m)

    kwargs = {}
    if TRACE:
        kwargs = dict(trace=True)
    res = run_bass_kernel_spmd(nc, in_maps, core_ids=list(range(NCORES)),
                               **kwargs)
    LAST_EXEC_NS = res.exec_time_ns
    LAST_RESULTS = res

    out = np.empty((B, S, D), dtype=np.float32)
    for b in range(B):
        acc = res.results[b * (NCORES // B)]["out2"].astype(np.float32)
        for g in range(1, NCORES // B):
            acc = acc + res.results[b * (NCORES // B) + g]["out2"]
        # out2 [128, DT, S] -> [D, S] -> [S, D]
        out[b] = acc.transpose(1, 0, 2).reshape(D, S).T
    return out
